# revision 1
# baseline (speedup 1.0000x reference)
"""Trainium2 Bass kernel for nn_AttentionBlock (GroupNorm + attention block),
data-parallel over batch across 8 NeuronCores.

Reference computation (per batch element b, C=512, N=H*W=1024, heads=8, hd=64):
  xn   = GroupNorm32(x) * gamma + beta
  qkv  = w_qkv @ xn + b_qkv        (1x1 conv == matmul over channels)
  attn = softmax(q^T k / sqrt(hd)) ; ha = attn @ v ; out = x + w_proj @ ha + b_proj

Sharding: batch B=8 -> one batch element per core. No collectives.

Per-core dataflow (matmuls in bf16 with f32 PSUM accumulation; weights are
passed as f32 and converted to bf16 on chip):
  - GroupNorm stats in f32: per-channel sum / sum-of-squares on DVE, group
    reduction + per-channel re-broadcast via tiny TensorE matmuls,
    rsqrt(var+eps) computed as exp(-0.5*ln(var+eps)) to stay in a single
    ScalarE table set (with the softmax Exp).
  - q,k produced in [channel, n] layout with host-pretransposed w_qkT.
  - v produced directly TRANSPOSED, v_T[n, c_v] = xn^T @ w_v^T (host
    pretransposed rhs), so attention needs no on-chip transposes. The v bias
    is folded in via a ones-row matmul accumulation. v_T is stored with
    head stride 65: 64 v columns + a ones column per head.
  - Scores computed transposed: S_T[m, n] = k_h^T q_h (K = hd = 64).
    Softmax without max subtraction (|scores*0.125| <~ 8, exp is safe in f32):
    P_T = exp(S_T * 0.125) on ScalarE directly out of PSUM (doubles as the
    PSUM eviction), written as bf16.
  - AV: ha_u[65, n] = [v_T | ones]^T @ P_T accumulated over the 8 m-chunks;
    row 64 is the softmax denominator Z. Normalization by 1/Z is applied at
    eviction: reciprocal on DVE, broadcast over partitions via a K=1 ones
    matmul on TensorE, multiply on DVE (writing bf16 for the proj matmul).
  - proj: out = (psum + b_proj) + x fused in one DVE scalar_tensor_tensor.
"""

import os

import numpy as np

import concourse.bass as bass
import concourse.bacc as bacc
import concourse.mybir as mybir
import concourse.tile as tile
from concourse.bass_utils import run_bass_kernel_spmd

F32 = mybir.dt.float32
BF16 = mybir.dt.bfloat16
AF = mybir.ActivationFunctionType
ALU = mybir.AluOpType

B = 8
C = 512
N = 1024          # H*W = 32*32
H = 8             # num heads
HD = 64           # head dim
G = 32            # groups
GS = C // G       # channels per group = 16
CCH = 4           # channel chunks of 128
NT = 2            # n tiles of 512
MT = 8            # m tiles of 128
EPS = 1e-5
P = 128
NCORES = 8

_CACHE = {}


def build_nc():
    nc = bacc.Bacc(
        "TRN2", target_bir_lowering=False, debug=False, num_devices=NCORES
    )

    # All parameters are 2-D float32, pre-arranged on the host so every DMA
    # below is a plain contiguous copy.
    x_d = nc.declare_dram_parameter("x", [C, N], F32, isOutput=False)
    wqk_d = nc.declare_dram_parameter("w_qkT", [C, 2 * C], F32, isOutput=False)
    bqk_d = nc.declare_dram_parameter("b_qk", [P, 8], F32, isOutput=False)
    wv_d = nc.declare_dram_parameter("w_vT", [C + 1, C], F32, isOutput=False)
    wp_d = nc.declare_dram_parameter("w_pT", [C, C], F32, isOutput=False)
    bp_d = nc.declare_dram_parameter("b_p", [P, CCH], F32, isOutput=False)
    gm_d = nc.declare_dram_parameter("gamma", [P, CCH], F32, isOutput=False)
    bt_d = nc.declare_dram_parameter("beta", [P, CCH], F32, isOutput=False)
    gsel_d = nc.declare_dram_parameter("gsel", [P, 8], F32, isOutput=False)
    gselT_d = nc.declare_dram_parameter("gselT", [8, P], F32, isOutput=False)
    out_d = nc.declare_dram_parameter("out", [C, N], F32, isOutput=True)

    with tile.TileContext(nc) as tc:
        with (
            tc.tile_pool(name="singles", bufs=1) as singles,
            tc.tile_pool(name="wstage", bufs=2) as wstage,
            tc.tile_pool(name="ps", bufs=2, space="PSUM") as ps_pool,
            tc.tile_pool(name="ps_av", bufs=2, space="PSUM") as ps_av_pool,
        ):
            # ---------------- static tiles ----------------
            x_sb = singles.tile([P, CCH, N], F32)
            wqk_sb = singles.tile([P, CCH, 2 * C], BF16)
            bqk_sb = singles.tile([P, 8], F32)
            wv_sb = singles.tile([P, CCH, C], BF16)
            wvb_sb = singles.tile([1, C], BF16)
            wp_sb = singles.tile([P, CCH, C], BF16)
            bp_sb = singles.tile([P, CCH], F32)
            gm_sb = singles.tile([P, CCH], F32)
            bt_sb = singles.tile([P, CCH], F32)
            gsel_sb = singles.tile([P, 8], BF16)
            gselT_sb = singles.tile([8, P], BF16)
            gsel_st = singles.tile([P, 8], F32)
            gselT_st = singles.tile([8, P], F32)
            s12_bf = singles.tile([P, 8], BF16)
            mu_rs_bf = singles.tile([8, 8], BF16)
            ones_row = singles.tile([1, P], BF16)
            ones64 = singles.tile([1, HD], BF16)

            xn_sb = singles.tile([P, CCH, N], BF16)
            qk_sb = singles.tile([P, 8, N], BF16)       # ot 0-3: q, 4-7: k
            vT_sb = singles.tile([P, MT, H * (HD + 16)], BF16)
            pT_a = singles.tile([P, MT, N], BF16)
            pT_b = singles.tile([P, MT, N], BF16)
            ha_sb = singles.tile([P, CCH, N], BF16)
            out_sb = singles.tile([P, CCH, N], F32)

            s12_sb = singles.tile([P, 8], F32)          # cols 0-3 sum, 4-7 sumsq
            sq_scr = singles.tile([P, N], F32)
            mu_rs = singles.tile([8, 8], F32)           # cols 0-3 mu, 4-7 rs
            ex2_sb = singles.tile([8, CCH], F32)
            tmp8 = singles.tile([8, CCH], F32)
            var_sb = singles.tile([8, CCH], F32)
            lnv_sb = singles.tile([8, CCH], F32)
            s0_sb = singles.tile([P, CCH], F32)
            sbias_sb = singles.tile([P, CCH], F32)
            tmp128 = singles.tile([P, CCH], F32)
            eps_sb = singles.tile([8, 1], F32)
            zinv_t = [
                singles.tile([16, N], BF16, name=f"zinv{i}") for i in range(2)
            ]
            zln_t = [
                singles.tile([16, N], F32, name=f"zln{i}") for i in range(2)
            ]
            ones16 = singles.tile([16, HD], BF16)
            zb_t = [singles.tile([HD, N], F32, name=f"zb{i}") for i in range(2)]

            # selector matrices DMA'd from host, converted to bf16 on chip
            nc.sync.dma_start(gsel_st[:], gsel_d.ap())
            nc.sync.dma_start(gselT_st[:], gselT_d.ap())
            nc.vector.tensor_copy(gsel_sb[:], gsel_st[:])
            nc.vector.tensor_copy(gselT_sb[:], gselT_st[:])
            nc.vector.memset(ones_row[:], 1.0)
            nc.vector.memset(ones64[:], 1.0)
            nc.vector.memset(eps_sb[:], EPS)
            nc.vector.memset(ones16[:], 1.0 / 16.0)

            # ---------------- input DMAs + weight bf16 conversion ----------
            x_v = x_d.ap().rearrange("(cc p) n -> p cc n", p=P)
            for cc in range(CCH):
                nc.sync.dma_start(x_sb[:, cc, :], x_v[:, cc, :])

            wqk_st = wstage.tile([P, CCH, 2 * C], F32, tag="wst")
            nc.sync.dma_start(
                wqk_st[:], wqk_d.ap().rearrange("(cc p) o -> p cc o", p=P)
            )
            nc.vector.tensor_copy(wqk_sb[:], wqk_st[:])

            wv_st = wstage.tile([P, CCH, C], F32, tag="wst")
            nc.sync.dma_start(
                wv_st[:], wv_d.ap()[0:C, :].rearrange("(cc p) v -> p cc v", p=P)
            )
            nc.vector.tensor_copy(wv_sb[:], wv_st[:])
            wvb_st = wstage.tile([1, C], F32, tag="wvbst")
            nc.sync.dma_start(wvb_st[:], wv_d.ap()[C : C + 1, :])
            nc.vector.tensor_copy(wvb_sb[:], wvb_st[:])

            wp_st = wstage.tile([P, CCH, C], F32, tag="wst")
            nc.sync.dma_start(
                wp_st[:], wp_d.ap().rearrange("(cc p) o -> p cc o", p=P)
            )
            nc.vector.tensor_copy(wp_sb[:], wp_st[:])

            nc.sync.dma_start(bqk_sb[:], bqk_d.ap())
            nc.sync.dma_start(bp_sb[:], bp_d.ap())
            nc.sync.dma_start(gm_sb[:], gm_d.ap())
            nc.sync.dma_start(bt_sb[:], bt_d.ap())

            # ---------------- GroupNorm stats ----------------
            for cc in range(CCH):
                nc.vector.reduce_sum(
                    s12_sb[:, cc : cc + 1], x_sb[:, cc, :], axis=mybir.AxisListType.X
                )
                nc.scalar.activation(
                    sq_scr[:], x_sb[:, cc, :], AF.Square,
                    accum_out=s12_sb[:, 4 + cc : 5 + cc],
                )
            # group reduce: [8 groups-in-chunk, 8 (s1 x cc, s2 x cc)]
            nc.vector.tensor_copy(s12_bf[:], s12_sb[:])
            ps_st = ps_pool.tile([P, N], F32, tag="ps")
            nc.tensor.matmul(
                ps_st[0:8, 0:8], gsel_sb[:], s12_bf[:], start=True, stop=True
            )
            inv_cnt = 1.0 / (GS * N)
            nc.vector.tensor_scalar_mul(mu_rs[:, 0:4], ps_st[0:8, 0:4], inv_cnt)
            nc.vector.tensor_scalar_mul(ex2_sb[:], ps_st[0:8, 4:8], inv_cnt)
            nc.vector.tensor_mul(tmp8[:], mu_rs[:, 0:4], mu_rs[:, 0:4])
            nc.vector.tensor_sub(var_sb[:], ex2_sb[:], tmp8[:])
            # rsqrt(var+eps) = exp(-0.5 * ln(var+eps)); keeps one ACT table set
            nc.scalar.activation(lnv_sb[:], var_sb[:], AF.Ln, bias=eps_sb[:])
            nc.scalar.activation(mu_rs[:, 4:8], lnv_sb[:], AF.Exp, scale=-0.5)
            # broadcast group stats back to channels
            nc.vector.tensor_copy(mu_rs_bf[:], mu_rs[:])
            ps_bc = ps_pool.tile([P, N], F32, tag="ps")
            nc.tensor.matmul(
                ps_bc[0:P, 0:8], gselT_sb[:], mu_rs_bf[:], start=True, stop=True
            )
            nc.vector.tensor_mul(s0_sb[:], ps_bc[0:P, 4:8], gm_sb[:])
            nc.vector.tensor_mul(tmp128[:], ps_bc[0:P, 0:4], s0_sb[:])
            nc.vector.tensor_sub(sbias_sb[:], bt_sb[:], tmp128[:])
            # xn = x * s0 + sbias  (bf16)
            for cc in range(CCH):
                nc.vector.tensor_scalar(
                    out=xn_sb[:, cc, :],
                    in0=x_sb[:, cc, :],
                    scalar1=s0_sb[:, cc : cc + 1],
                    scalar2=sbias_sb[:, cc : cc + 1],
                    op0=ALU.mult,
                    op1=ALU.add,
                )

            # ---------------- q, k ----------------
            for ot in range(8):
                ps_qk = ps_pool.tile([P, N], F32, tag="ps")
                for nt in range(NT):
                    for cc in range(CCH):
                        nc.tensor.matmul(
                            ps_qk[:, nt * 512 : (nt + 1) * 512],
                            wqk_sb[:, cc, ot * P : (ot + 1) * P],
                            xn_sb[:, cc, nt * 512 : (nt + 1) * 512],
                            start=(cc == 0),
                            stop=(cc == CCH - 1),
                        )
                nc.scalar.activation(
                    qk_sb[:, ot, :], ps_qk[:], AF.Identity,
                    bias=bqk_sb[:, ot : ot + 1],
                )

            # ---------------- v_T (+ bias via ones row) ----------------
            for mt in range(MT):
                ps_v = ps_pool.tile([P, N], F32, tag="ps")
                for cc in range(CCH):
                    nc.tensor.matmul(
                        ps_v[:, 0:C],
                        xn_sb[:, cc, mt * P : (mt + 1) * P],
                        wv_sb[:, cc, :],
                        start=(cc == 0),
                        stop=False,
                    )
                nc.tensor.matmul(
                    ps_v[:, 0:C], ones_row[:], wvb_sb[:], start=False, stop=True
                )
                nc.vector.tensor_copy(
                    vT_sb[:, mt, :]
                    .rearrange("p (h d) -> p h d", h=H)[:, :, 0:HD],
                    ps_v[:, 0:C].rearrange("p (h d) -> p h d", h=H),
                )
            nc.vector.memset(
                vT_sb[:].rearrange("p mt (h d) -> p mt h d", h=H)[:, :, :, HD : HD + 16],
                1.0,
            )

            # ---------------- attention, software-pipelined over heads ----
            # PE program order: S_T(h) ... AV(h-1) ... so AV's dependency
            # (exp of head h-1 on ScalarE) hides behind S_T(h)'s matmuls.
            def st_exp(h):
                pT = pT_a if h % 2 == 0 else pT_b
                po = (h % 2) * HD
                ot = h // 2
                for mt in range(MT):
                    ps_s = ps_pool.tile([P, N], F32, tag="ps", name=f"ps_s{h}_{mt}")
                    for nt in range(NT):
                        nc.tensor.matmul(
                            ps_s[:, nt * 512 : (nt + 1) * 512],
                            qk_sb[po : po + HD, 4 + ot, mt * P : (mt + 1) * P],
                            qk_sb[po : po + HD, ot, nt * 512 : (nt + 1) * 512],
                            start=True,
                            stop=True,
                        )
                    nc.scalar.activation(
                        pT[:, mt, :], ps_s[:], AF.Exp, scale=float(HD) ** -0.5
                    )

            def av_evict(h):
                pT = pT_a if h % 2 == 0 else pT_b
                po = (h % 2) * HD
                ot = h // 2
                zinv = zinv_t[h % 2]
                zln = zln_t[h % 2]
                zb = zb_t[h % 2]
                W = HD + 16
                ps_av = ps_av_pool.tile([P, N], F32, tag="av", name=f"ps_av{h}")
                for nt in range(NT):
                    for mt in range(MT):
                        nc.tensor.matmul(
                            ps_av[0:W, nt * 512 : (nt + 1) * 512],
                            vT_sb[:, mt, h * W : (h + 1) * W],
                            pT[:, mt, nt * 512 : (nt + 1) * 512],
                            start=(mt == 0),
                            stop=(mt == MT - 1),
                        )
                nc.vector.reciprocal(zln[:], ps_av[HD:W, :])
                nc.vector.tensor_copy(zinv[:], zln[:])
                ps_zb = ps_pool.tile([P, N], F32, tag="ps", name=f"ps_zb{h}")
                for nt in range(NT):
                    nc.tensor.matmul(
                        ps_zb[0:HD, nt * 512 : (nt + 1) * 512],
                        ones16[:],
                        zinv[:, nt * 512 : (nt + 1) * 512],
                        start=True,
                        stop=True,
                    )
                nc.vector.tensor_copy(zb[:], ps_zb[0:HD, :])
                nc.vector.tensor_mul(
                    ha_sb[po : po + HD, ot, :], ps_av[0:HD, :], zb[:]
                )

            st_exp(0)
            for h in range(1, H):
                st_exp(h)
                av_evict(h - 1)
            av_evict(H - 1)

            # ---------------- proj + bias + residual ----------------
            out_v = out_d.ap().rearrange("(ot p) n -> p ot n", p=P)
            for ot in range(CCH):
                ps_p = ps_pool.tile([P, N], F32, tag="ps")
                for nt in range(NT):
                    for cc in range(CCH):
                        nc.tensor.matmul(
                            ps_p[:, nt * 512 : (nt + 1) * 512],
                            wp_sb[:, cc, ot * P : (ot + 1) * P],
                            ha_sb[:, cc, nt * 512 : (nt + 1) * 512],
                            start=(cc == 0),
                            stop=(cc == CCH - 1),
                        )
                nc.vector.scalar_tensor_tensor(
                    out=out_sb[:, ot, :],
                    in0=ps_p[:],
                    scalar=bp_sb[:, ot : ot + 1],
                    in1=x_sb[:, ot, :],
                    op0=ALU.add,
                    op1=ALU.add,
                )
                nc.sync.dma_start(out_v[:, ot, :], out_sb[:, ot, :])

    nc.compile()
    return nc


def make_in_maps(x, gn_gamma, gn_beta, w_qkv, b_qkv, w_proj, b_proj):
    f32 = np.float32
    w_qkv = np.asarray(w_qkv, dtype=f32)
    b_qkv = np.asarray(b_qkv, dtype=f32)
    shared = {
        "w_qkT": np.ascontiguousarray(w_qkv[: 2 * C].T),
        "b_qk": np.ascontiguousarray(b_qkv[: 2 * C].reshape(8, P).T),
        "w_vT": np.ascontiguousarray(
            np.concatenate([w_qkv[2 * C :].T, b_qkv[2 * C :][None, :]], axis=0)
        ),
        "w_pT": np.ascontiguousarray(np.asarray(w_proj, dtype=f32).T),
        "b_p": np.ascontiguousarray(
            np.asarray(b_proj, dtype=f32).reshape(CCH, P).T
        ),
        "gamma": np.ascontiguousarray(
            np.asarray(gn_gamma, dtype=f32).reshape(CCH, P).T
        ),
        "beta": np.ascontiguousarray(
            np.asarray(gn_beta, dtype=f32).reshape(CCH, P).T
        ),
    }
    gsel = np.zeros((P, 8), f32)
    for p in range(P):
        gsel[p, p // GS] = 1.0
    shared["gsel"] = gsel
    shared["gselT"] = np.ascontiguousarray(gsel.T)
    in_maps = []
    for b in range(B):
        m = dict(shared)
        m["x"] = np.ascontiguousarray(np.asarray(x[b], dtype=f32).reshape(C, N))
        in_maps.append(m)
    return in_maps


def kernel(x, gn_gamma, gn_beta, w_qkv, b_qkv, w_proj, b_proj):
    if "nc" not in _CACHE:
        _CACHE["nc"] = build_nc()
    nc = _CACHE["nc"]
    in_maps = make_in_maps(x, gn_gamma, gn_beta, w_qkv, b_qkv, w_proj, b_proj)
    trace = bool(os.environ.get("KERNEL_TRACE"))
    res = run_bass_kernel_spmd(
        nc, in_maps, core_ids=list(range(NCORES)), trace=trace
    )
    _CACHE["last_result"] = res
    out = np.stack([np.asarray(res.results[i]["out"]) for i in range(NCORES)])
    return out.reshape(B, C, 32, 32).astype(np.float32)



# revision 11
# speedup vs baseline: 1.1215x; 1.1215x over previous
"""Trainium2 Bass kernel for nn_AttentionBlock (GroupNorm + attention block),
data-parallel over batch across 8 NeuronCores.

Reference computation (per batch element b, C=512, N=H*W=1024, heads=8, hd=64):
  xn   = GroupNorm32(x) * gamma + beta
  qkv  = w_qkv @ xn + b_qkv        (1x1 conv == matmul over channels)
  attn = softmax(q^T k / sqrt(hd)) ; ha = attn @ v ; out = x + w_proj @ ha + b_proj
Sharding: batch B=8 -> one batch element per core. No collectives.

v3 design notes (per-core; ScalarE runs ONLY the softmax exp -> one ACT
table set, loaded once via a dummy exp during the input DMA):
  - Weights pre-transposed and pre-cast to bf16 on the host; contiguous DMAs
    spread across three hardware DGE queues (sync: x, scalar: weights,
    gpsimd: params) so the prologue is not serialized on one queue.
  - k bias dropped (softmax shift-invariance along keys); only q keeps its
    bias, applied on DVE at PSUM eviction.
  - GroupNorm: per-channel sum via DVE reduce, sumsq via DVE
    scalar_tensor_tensor (x*1)*x with accum_out; group reduce/broadcast via
    tiny TensorE matmuls; rsqrt(var+eps) via int-shift seed (0x5F3759DF) + 2
    float Newton steps, all standard DVE ops (keeps Ln/Sqrt sets off ACT).
  - Heads in PAIRS: head 2p at partitions 0-63, 2p+1 at 64-127 of the shared
    q/k tiles; the two K=64 S_T matmuls land in disjoint PE row groups
    (tile_position auto-derived) and can run concurrently.
  - P_T = exp(S_T/8) on ScalarE straight out of PSUM (no max subtraction;
    |scores/8| <~ 8). 64 x ~1.1us of exp is the steady-state bottleneck.
  - AV: ha_u[80, n] = [v_T | ones16]^T @ P_T accumulated mt-outer (LDWEIGHTS
    shared across the nt halves); rows 64-79 hold the denominator Z.
  - 1/Z: RECIPROCAL_APPROX_FAST's algorithm decomposed into standard DVE ops
    (bitwise_not seed, Chebyshev pair c0/c1, one fused NR step):
      w1 = (Z*(~Z)*c0 - c1) * (~Z);  1/Z ~= -c0*w1   (~0.2% rel err)
    with -c0/16 folded into the zb broadcast selector. All tiles live on
    partitions 64-79 to stay aligned with the PSUM Z rows.
  - zb broadcast via K=16 selector matmul to 64 partitions; ha = ha_u*zb.
  - PE filler between S_T chunks: v / next-pair qk / prev-pair AV / zb.
  - proj: (psum + b_proj) + x fused in one DVE scalar_tensor_tensor; first
    two output tiles defer their last cc chunk so proj matmuls cover the
    final pair's normalization chain.
"""

import os

import numpy as np
import ml_dtypes

import concourse.bass as bass
import concourse.bacc as bacc
import concourse.mybir as mybir
import concourse.tile as tile
from concourse.bass_utils import run_bass_kernel_spmd

F32 = mybir.dt.float32
BF16 = mybir.dt.bfloat16
I32 = mybir.dt.int32
AF = mybir.ActivationFunctionType
ALU = mybir.AluOpType

B = 8
C = 512
N = 1024          # H*W = 32*32
H = 8             # num heads
HD = 64           # head dim
G = 32            # groups
GS = C // G       # channels per group = 16
CCH = 4           # channel chunks of 128
NT = 2            # n tiles of 512
MT = 8            # m tiles of 128
EPS = 1e-5
P = 128
NCORES = 8
W80 = HD + 16     # v columns + 16 ones columns per head

RC0 = -0.23549792          # Chebyshev recip seed scale
RC1 = 2.0017324
RSQRT_MAGIC = 0x5F3759DF

_CACHE = {}


def build_nc():
    nc = bacc.Bacc(
        "TRN2", target_bir_lowering=False, debug=False, num_devices=NCORES
    )

    x_d = nc.declare_dram_parameter("x", [C, N], F32, isOutput=False)
    wqk_d = nc.declare_dram_parameter("w_qkT", [P, CCH * 2 * C], BF16, isOutput=False)
    wv_d = nc.declare_dram_parameter("w_vT", [P, CCH * C], BF16, isOutput=False)
    wvb_d = nc.declare_dram_parameter("w_vb", [1, C], BF16, isOutput=False)
    wp_d = nc.declare_dram_parameter("w_pT", [P, CCH * C], BF16, isOutput=False)
    pf_d = nc.declare_dram_parameter("pf32", [P, 16], F32, isOutput=False)
    pb_d = nc.declare_dram_parameter("pbf16", [P, 200], BF16, isOutput=False)
    out_d = nc.declare_dram_parameter("out", [C, N], F32, isOutput=True)

    with tile.TileContext(nc) as tc:
        with (
            tc.tile_pool(name="singles", bufs=1) as singles,
            tc.tile_pool(name="outbuf", bufs=2) as outbuf,
            tc.tile_pool(name="ps", bufs=2, space="PSUM") as ps_pool,
            tc.tile_pool(name="ps_av", bufs=2, space="PSUM") as ps_av_pool,
        ):
            # ---------------- static tiles ----------------
            x_sb = singles.tile([P, CCH, N], F32)
            xn_sb = singles.tile([P, CCH, N], BF16)
            wqk_sb = singles.tile([P, CCH, 2 * C], BF16)
            wv_sb = singles.tile([P, CCH, C], BF16)
            wvb_sb = singles.tile([1, C], BF16)
            wp_sb = singles.tile([P, CCH, C], BF16)
            pf_sb = singles.tile([P, 16], F32)    # bq(4) bp(4) gamma(4) beta(4)
            pb_sb = singles.tile([P, 200], BF16)  # gsel(8) gselT(128) zsel(64)

            qk_sb = singles.tile([P, 8, N], BF16)       # ot 0-3: q, 4-7: k
            vT_sb = singles.tile([P, MT, H * W80], BF16)
            pT_t = [
                singles.tile([P, MT, N], BF16, name=f"pT{i}") for i in range(4)
            ]
            ha_sb = singles.tile([P, CCH, N], BF16)
            ones_row = singles.tile([1, P], BF16)

            # GroupNorm scratch
            s12_sb = singles.tile([P, 8], F32)          # cols 0-3 sum, 4-7 sumsq
            s12_bf = singles.tile([P, 8], BF16)
            sq_scr = singles.tile([P, N], F32)
            mu_rs = singles.tile([8, 8], F32)           # cols 0-3 mu, 4-7 rs
            mu_rs_bf = singles.tile([8, 8], BF16)
            ex2_sb = singles.tile([8, CCH], F32)
            tmp8 = singles.tile([8, CCH], F32)
            var_sb = singles.tile([8, CCH], F32)        # then var+eps
            rsq_i = singles.tile([8, CCH], I32)         # int seed scratch
            rsq_r = singles.tile([8, CCH], F32)         # rsqrt iterate
            rsq_t = singles.tile([8, CCH], F32)
            rsq_u = singles.tile([8, CCH], F32)
            s0_sb = singles.tile([P, CCH], F32)
            sbias_sb = singles.tile([P, CCH], F32)
            tmp128 = singles.tile([P, CCH], F32)

            # softmax 1/Z scratch: rows 64-79 only (aligned with PSUM Z rows);
            # col range [0, N) = head A of the pair, [N, 2N) = head B.
            zw_sb = singles.tile([P, 2 * N], F32)       # ~Z seed (bits)
            zt_sb = singles.tile([P, 2 * N], F32)       # Z * y0
            zi_sb = singles.tile([P, 2 * N], BF16)      # w1 (recip * -1/c0)
            zb_t = [singles.tile([HD, N], F32, name=f"zb{i}") for i in range(4)]

            # ---------------- input DMAs (3 queues) ----------------
            x_v = x_d.ap().rearrange("(cc p) n -> p cc n", p=P)
            for cc in range(CCH):
                nc.sync.dma_start(x_sb[:, cc, :], x_v[:, cc, :])
            nc.scalar.dma_start(
                wqk_sb[:], wqk_d.ap().rearrange("p (cc o) -> p cc o", cc=CCH)
            )
            nc.scalar.dma_start(
                wv_sb[:], wv_d.ap().rearrange("p (cc o) -> p cc o", cc=CCH)
            )
            nc.scalar.dma_start(wvb_sb[:], wvb_d.ap())
            nc.scalar.dma_start(
                wp_sb[:], wp_d.ap().rearrange("p (cc o) -> p cc o", cc=CCH)
            )
            nc.gpsimd.dma_start(pf_sb[:], pf_d.ap())
            nc.gpsimd.dma_start(pb_sb[:], pb_d.ap())

            bq = pf_sb[:, 0:4]
            bp = pf_sb[:, 4:8]
            gm = pf_sb[:, 8:12]
            bt = pf_sb[:, 12:16]
            gsel = pb_sb[:, 0:8]
            gselT = pb_sb[0:8, 8:136]
            zsel = pb_sb[64:80, 136:200]   # [16, 64] = -c0/16

            nc.vector.memset(ones_row[:], 1.0)
            nc.vector.memset(
                vT_sb[:].rearrange("p mt (h d) -> p mt h d", h=H)[:, :, :, HD:W80],
                1.0,
            )
            # Preload the exp ACT table set while the input DMAs run.
            nc.scalar.activation(sq_scr[0:1, 0:P], ones_row[:], AF.Exp)

            # ---------------- GroupNorm stats ----------------
            for cc in range(CCH):
                nc.vector.reduce_sum(
                    s12_sb[:, cc : cc + 1], x_sb[:, cc, :], axis=mybir.AxisListType.X
                )
                nc.vector.scalar_tensor_tensor(
                    out=sq_scr[:],
                    in0=x_sb[:, cc, :],
                    scalar=1.0,
                    in1=x_sb[:, cc, :],
                    op0=ALU.mult,
                    op1=ALU.mult,
                    accum_out=s12_sb[:, 4 + cc : 5 + cc],
                )
            nc.vector.tensor_copy(s12_bf[:], s12_sb[:])
            ps_st = ps_pool.tile([P, N], F32, tag="ps")
            nc.tensor.matmul(
                ps_st[0:8, 0:8], gsel, s12_bf[:], start=True, stop=True
            )
            inv_cnt = 1.0 / (GS * N)
            nc.vector.tensor_scalar_mul(mu_rs[:, 0:4], ps_st[0:8, 0:4], inv_cnt)
            nc.vector.tensor_scalar_mul(ex2_sb[:], ps_st[0:8, 4:8], inv_cnt)
            nc.vector.tensor_mul(tmp8[:], mu_rs[:, 0:4], mu_rs[:, 0:4])
            nc.vector.tensor_sub(var_sb[:], ex2_sb[:], tmp8[:])
            nc.vector.tensor_scalar_add(var_sb[:], var_sb[:], EPS)
            # rsqrt(var+eps): int seed MAGIC - (bits >> 1), then 2 Newton steps
            nc.vector.tensor_scalar(
                out=rsq_i[:], in0=var_sb[:].bitcast(I32),
                scalar1=1, scalar2=None, op0=ALU.arith_shift_right,
            )
            nc.vector.tensor_scalar_sub(rsq_i[:], rsq_i[:], RSQRT_MAGIC)
            nc.vector.tensor_scalar(            # ~x (then +1 below: -x = ~x+1)
                out=rsq_i[:], in0=rsq_i[:],
                scalar1=0, scalar2=None, op0=ALU.bitwise_not,
            )
            nc.vector.tensor_scalar_add(rsq_r[:].bitcast(I32), rsq_i[:], 1)
            for _ in range(2):
                nc.vector.tensor_mul(rsq_t[:], rsq_r[:], rsq_r[:])
                nc.vector.scalar_tensor_tensor(
                    out=rsq_u[:], in0=rsq_t[:], scalar=-0.5, in1=var_sb[:],
                    op0=ALU.mult, op1=ALU.mult,
                )
                nc.vector.scalar_tensor_tensor(
                    out=rsq_r[:], in0=rsq_u[:], scalar=1.5, in1=rsq_r[:],
                    op0=ALU.add, op1=ALU.mult,
                )
            nc.vector.tensor_copy(mu_rs[:, 4:8], rsq_r[:])
            nc.vector.tensor_copy(mu_rs_bf[:], mu_rs[:])
            ps_bc = ps_pool.tile([P, N], F32, tag="ps")
            nc.tensor.matmul(
                ps_bc[0:P, 0:8], gselT, mu_rs_bf[:], start=True, stop=True
            )
            nc.vector.tensor_mul(s0_sb[:], ps_bc[0:P, 4:8], gm)
            nc.vector.tensor_mul(tmp128[:], ps_bc[0:P, 0:4], s0_sb[:])
            nc.vector.tensor_sub(sbias_sb[:], bt, tmp128[:])
            for cc in range(CCH):
                nc.vector.tensor_scalar(
                    out=xn_sb[:, cc, :],
                    in0=x_sb[:, cc, :],
                    scalar1=s0_sb[:, cc : cc + 1],
                    scalar2=sbias_sb[:, cc : cc + 1],
                    op0=ALU.mult,
                    op1=ALU.add,
                )

            # ---------------- emission helpers ----------------
            def qk_ot(ot):
                """q (ot<4) or k (ot>=4) output tile: 8 matmuls + eviction."""
                ps_qk = ps_pool.tile([P, N], F32, tag="ps", name=f"qk{ot}")
                for cc in range(CCH):
                    for nt in range(NT):
                        nc.tensor.matmul(
                            ps_qk[:, nt * 512 : (nt + 1) * 512],
                            wqk_sb[:, cc, ot * P : (ot + 1) * P],
                            xn_sb[:, cc, nt * 512 : (nt + 1) * 512],
                            start=(cc == 0),
                            stop=(cc == CCH - 1),
                        )
                if ot < 4:  # q: add bias on eviction
                    nc.vector.tensor_scalar_add(
                        qk_sb[:, ot, :], ps_qk[:], bq[:, ot : ot + 1]
                    )
                else:       # k: bias cancels in softmax; plain copy
                    nc.vector.tensor_copy(qk_sb[:, ot, :], ps_qk[:])

            def v_mt(mt):
                """v_T chunk for rows [128*mt, 128*mt+128): 5 matmuls + copy."""
                ps_v = ps_pool.tile([P, N], F32, tag="ps", name=f"v{mt}")
                for cc in range(CCH):
                    nc.tensor.matmul(
                        ps_v[:, 0:C],
                        xn_sb[:, cc, mt * P : (mt + 1) * P],
                        wv_sb[:, cc, :],
                        start=(cc == 0),
                        stop=False,
                    )
                nc.tensor.matmul(
                    ps_v[:, 0:C], ones_row[:], wvb_sb[:], start=False, stop=True
                )
                nc.vector.tensor_copy(
                    vT_sb[:, mt, :]
                    .rearrange("p (h d) -> p h d", h=H)[:, :, 0:HD],
                    ps_v[:, 0:C].rearrange("p (h d) -> p h d", h=H),
                )

            def st_pair_mt(pr, mt):
                """S_T + exp for heads (2pr, 2pr+1), m-chunk mt; disjoint PE
                row groups -> the two heads' matmuls can run concurrently."""
                ot = pr
                pa = pT_t[(pr % 2) * 2]
                pb2 = pT_t[(pr % 2) * 2 + 1]
                psA = ps_pool.tile([P, N], F32, tag="ps", name=f"sA{pr}_{mt}")
                psB = ps_pool.tile([P, N], F32, tag="ps", name=f"sB{pr}_{mt}")
                for nt in range(NT):
                    sl = slice(nt * 512, (nt + 1) * 512)
                    nc.tensor.matmul(
                        psA[:, sl],
                        qk_sb[0:HD, 4 + ot, mt * P : (mt + 1) * P],
                        qk_sb[0:HD, ot, sl],
                        start=True, stop=True,
                    )
                    nc.tensor.matmul(
                        psB[:, sl],
                        qk_sb[HD:P, 4 + ot, mt * P : (mt + 1) * P],
                        qk_sb[HD:P, ot, sl],
                        start=True, stop=True,
                    )
                scale = float(HD) ** -0.5
                nc.scalar.activation(pa[:, mt, :], psA[:], AF.Exp, scale=scale)
                nc.scalar.activation(pb2[:, mt, :], psB[:], AF.Exp, scale=scale)

            def av_mt(pr, mt, ps_avA, ps_avB):
                """AV accumulation chunk mt for head pair pr (4 matmuls)."""
                hA, hB = 2 * pr, 2 * pr + 1
                pa = pT_t[(pr % 2) * 2]
                pb2 = pT_t[(pr % 2) * 2 + 1]
                for (h, pt, ps_av) in ((hA, pa, ps_avA), (hB, pb2, ps_avB)):
                    for nt in range(NT):
                        sl = slice(nt * 512, (nt + 1) * 512)
                        nc.tensor.matmul(
                            ps_av[0:W80, sl],
                            vT_sb[:, mt, h * W80 : (h + 1) * W80],
                            pt[:, mt, sl],
                            start=(mt == 0),
                            stop=(mt == MT - 1),
                        )

            def norm_pair_dve(pr, ps_avA, ps_avB):
                """w1 = (Z*~Z*c0 - c1)*~Z per head (fast recip minus the final
                scale, folded into zsel). Standard DVE ops only."""
                for (ps_av, off) in ((ps_avA, 0), (ps_avB, N)):
                    zr = ps_av[HD:W80, :]
                    sl = slice(off, off + N)
                    nc.vector.tensor_scalar(
                        out=zw_sb[HD:W80, sl].bitcast(I32),
                        in0=zr.bitcast(I32),
                        scalar1=0, scalar2=None, op0=ALU.bitwise_not,
                    )
                    nc.vector.scalar_tensor_tensor(   # zt = (~Z * c0) * Z
                        out=zt_sb[HD:W80, sl],
                        in0=zw_sb[HD:W80, sl],
                        scalar=RC0,
                        in1=zr,
                        op0=ALU.mult, op1=ALU.mult,
                    )
                nc.vector.scalar_tensor_tensor(       # w1 = (zt - c1) * ~Z
                    out=zi_sb[HD:W80, :],
                    in0=zt_sb[HD:W80, :],
                    scalar=RC1,
                    in1=zw_sb[HD:W80, :],
                    op0=ALU.subtract, op1=ALU.mult,
                )

            def norm_pair_pe(pr, ps_avA, ps_avB):
                """zb = broadcast(1/Z) to 64 partitions (selector carries the
                -c0/16 scale), then ha = ha_u * zb on DVE."""
                ot = pr
                zbA = zb_t[(pr % 2) * 2]
                zbB = zb_t[(pr % 2) * 2 + 1]
                ps_zbA = ps_pool.tile([P, N], F32, tag="ps", name=f"zbA{pr}")
                ps_zbB = ps_pool.tile([P, N], F32, tag="ps", name=f"zbB{pr}")
                for nt in range(NT):
                    sl = slice(nt * 512, (nt + 1) * 512)
                    nc.tensor.matmul(
                        ps_zbA[0:HD, sl], zsel, zi_sb[HD:W80, sl],
                        start=True, stop=True,
                    )
                    nc.tensor.matmul(
                        ps_zbB[0:HD, sl], zsel,
                        zi_sb[HD:W80, N + sl.start : N + sl.stop],
                        start=True, stop=True,
                    )
                nc.vector.tensor_copy(zbA[:], ps_zbA[0:HD, :])
                nc.vector.tensor_copy(zbB[:], ps_zbB[0:HD, :])
                nc.vector.tensor_mul(ha_sb[0:HD, ot, :], ps_avA[0:HD, :], zbA[:])
                nc.vector.tensor_mul(ha_sb[HD:P, ot, :], ps_avB[0:HD, :], zbB[:])

            def proj_ot_mms(ot, ps_p, ccs):
                for cc in ccs:
                    for nt in range(NT):
                        nc.tensor.matmul(
                            ps_p[:, nt * 512 : (nt + 1) * 512],
                            wp_sb[:, cc, ot * P : (ot + 1) * P],
                            ha_sb[:, cc, nt * 512 : (nt + 1) * 512],
                            start=(cc == 0),
                            stop=(cc == CCH - 1),
                        )

            # ---------------- prologue: q0/k0 ----------------
            qk_ot(0)
            qk_ot(4)

            # ---------------- head-pair pipeline ----------------
            # Step 0: S_T/exp of pair 0, with v chunks and q1/k1 as PE filler.
            for mt in range(MT):
                st_pair_mt(0, mt)
                v_mt(mt)
                if mt == 2:
                    qk_ot(1)
                if mt == 5:
                    qk_ot(5)

            # Steps 1-3: S_T/exp of pair pr overlapped with AV of pair pr-1
            # (AV one chunk ahead so Z completes while S_T still has work).
            for pr in range(1, 4):
                avA = ps_av_pool.tile([P, N], F32, tag="av", name=f"avA{pr-1}")
                avB = ps_av_pool.tile([P, N], F32, tag="av", name=f"avB{pr-1}")
                st_pair_mt(pr, 0)
                av_mt(pr - 1, 0, avA, avB)
                av_mt(pr - 1, 1, avA, avB)
                for mt in range(1, MT - 1):
                    st_pair_mt(pr, mt)
                    av_mt(pr - 1, mt + 1, avA, avB)
                    if mt == MT - 2:
                        norm_pair_dve(pr - 1, avA, avB)
                    if pr < 3:
                        if mt == 1:
                            qk_ot(pr + 1)
                        if mt == 3:
                            qk_ot(pr + 5)
                st_pair_mt(pr, MT - 1)
                norm_pair_pe(pr - 1, avA, avB)

            # Final step: AV of pair 3, normalization, then proj with the
            # first two output tiles deferring their last cc chunk.
            avA = ps_av_pool.tile([P, N], F32, tag="av", name="avA3")
            avB = ps_av_pool.tile([P, N], F32, tag="av", name="avB3")
            for mt in range(MT):
                av_mt(3, mt, avA, avB)
            norm_pair_dve(3, avA, avB)
            norm_pair_pe(3, avA, avB)

            # ---------------- proj + bias + residual ----------------
            out_v = out_d.ap().rearrange("(ot p) n -> p ot n", p=P)

            def proj_evict(ot, ps_p):
                out_t = outbuf.tile([P, N], F32, tag="out")
                nc.vector.scalar_tensor_tensor(
                    out=out_t[:],
                    in0=ps_p[:],
                    scalar=bp[:, ot : ot + 1],
                    in1=x_sb[:, ot, :],
                    op0=ALU.add,
                    op1=ALU.add,
                )
                nc.sync.dma_start(out_v[:, ot, :], out_t[:])

            ps_p0 = ps_pool.tile([P, N], F32, tag="ps", name="proj0")
            proj_ot_mms(0, ps_p0, range(CCH - 1))
            ps_p1 = ps_pool.tile([P, N], F32, tag="ps", name="proj1")
            proj_ot_mms(1, ps_p1, range(CCH - 1))
            proj_ot_mms(0, ps_p0, [CCH - 1])
            proj_evict(0, ps_p0)
            proj_ot_mms(1, ps_p1, [CCH - 1])
            proj_evict(1, ps_p1)
            for ot in range(2, CCH):
                ps_p = ps_pool.tile([P, N], F32, tag="ps", name=f"proj{ot}")
                proj_ot_mms(ot, ps_p, range(CCH))
                proj_evict(ot, ps_p)

    nc.compile()
    return nc


def make_in_maps(x, gn_gamma, gn_beta, w_qkv, b_qkv, w_proj, b_proj):
    f32 = np.float32
    bf16 = ml_dtypes.bfloat16
    w_qkv = np.asarray(w_qkv, dtype=f32)
    b_qkv = np.asarray(b_qkv, dtype=f32)

    def chunked_T(w):
        # [O, C_in] -> transposed [C_in, O] -> SBUF layout [p, cc, O] packed
        # as [P, cc*O] with in-channel c = cc*128 + p.
        wt = np.ascontiguousarray(w.T, dtype=f32)  # [C_in, O]
        o = wt.shape[1]
        return np.ascontiguousarray(
            wt.reshape(CCH, P, o).transpose(1, 0, 2).reshape(P, CCH * o)
        ).astype(bf16)

    def perch(v):
        return np.asarray(v, dtype=f32).reshape(CCH, P).T  # [P, CCH]

    pf32 = np.concatenate(
        [
            perch(b_qkv[:C]),          # q bias
            perch(np.asarray(b_proj, dtype=f32)),
            perch(np.asarray(gn_gamma, dtype=f32)),
            perch(np.asarray(gn_beta, dtype=f32)),
        ],
        axis=1,
    ).astype(f32)

    # pbf16: gsel [P, 0:8], gselT [rows 0-7, 8:136], zsel [rows 64-79, 136:200]
    gsel = np.zeros((P, 8), f32)
    for p in range(P):
        gsel[p, p // GS] = 1.0
    pbf16 = np.zeros((P, 200), f32)
    pbf16[:, 0:8] = gsel
    pbf16[0:8, 8:136] = gsel.T
    pbf16[64:80, 136:200] = -RC0 / 16.0
    pbf16 = np.ascontiguousarray(pbf16).astype(bf16)

    shared = {
        "w_qkT": chunked_T(w_qkv[: 2 * C]),
        "w_vT": chunked_T(w_qkv[2 * C :]),
        "w_vb": np.ascontiguousarray(b_qkv[2 * C :][None, :]).astype(bf16),
        "w_pT": chunked_T(np.asarray(w_proj, dtype=f32)),
        "pf32": pf32,
        "pbf16": pbf16,
    }
    in_maps = []
    for b in range(B):
        m = dict(shared)
        m["x"] = np.ascontiguousarray(np.asarray(x[b], dtype=f32).reshape(C, N))
        in_maps.append(m)
    return in_maps


def kernel(x, gn_gamma, gn_beta, w_qkv, b_qkv, w_proj, b_proj):
    if "nc" not in _CACHE:
        _CACHE["nc"] = build_nc()
    nc = _CACHE["nc"]
    in_maps = make_in_maps(x, gn_gamma, gn_beta, w_qkv, b_qkv, w_proj, b_proj)
    trace = bool(os.environ.get("KERNEL_TRACE"))
    res = run_bass_kernel_spmd(
        nc, in_maps, core_ids=list(range(NCORES)), trace=trace
    )
    _CACHE["last_result"] = res
    out = np.stack([np.asarray(res.results[i]["out"]) for i in range(NCORES)])
    return out.reshape(B, C, 32, 32).astype(np.float32)


# revision 13
# speedup vs baseline: 1.1915x; 1.0624x over previous
"""Trainium2 Bass kernel for nn_AttentionBlock (GroupNorm + attention block),
data-parallel over batch across 8 NeuronCores.

Reference computation (per batch element b, C=512, N=H*W=1024, heads=8, hd=64):
  xn   = GroupNorm32(x) * gamma + beta
  qkv  = w_qkv @ xn + b_qkv        (1x1 conv == matmul over channels)
  attn = softmax(q^T k / sqrt(hd)) ; ha = attn @ v ; out = x + w_proj @ ha + b_proj
Sharding: batch B=8 -> one batch element per core. No collectives.

v3 design notes (per-core; ScalarE runs ONLY the softmax exp -> one ACT
table set, loaded once via a dummy exp during the input DMA):
  - Weights pre-transposed and pre-cast to bf16 on the host; contiguous DMAs
    spread across three hardware DGE queues (sync: x, scalar: weights,
    gpsimd: params) so the prologue is not serialized on one queue.
  - k bias dropped (softmax shift-invariance along keys); only q keeps its
    bias, applied on DVE at PSUM eviction.
  - GroupNorm: per-channel sum via DVE reduce, sumsq via DVE
    scalar_tensor_tensor (x*1)*x with accum_out; group reduce/broadcast via
    tiny TensorE matmuls; rsqrt(var+eps) via int-shift seed (0x5F3759DF) + 2
    float Newton steps, all standard DVE ops (keeps Ln/Sqrt sets off ACT).
  - Heads in PAIRS: head 2p at partitions 0-63, 2p+1 at 64-127 of the shared
    q/k tiles; the two K=64 S_T matmuls land in disjoint PE row groups
    (tile_position auto-derived) and can run concurrently.
  - P_T = exp(S_T/8) on ScalarE straight out of PSUM (no max subtraction;
    |scores/8| <~ 8). 64 x ~1.1us of exp is the steady-state bottleneck.
  - AV: ha_u[80, n] = [v_T | ones16]^T @ P_T accumulated mt-outer (LDWEIGHTS
    shared across the nt halves); rows 64-79 hold the denominator Z.
  - 1/Z: RECIPROCAL_APPROX_FAST's algorithm decomposed into standard DVE ops
    (bitwise_not seed, Chebyshev pair c0/c1, one fused NR step):
      w1 = (Z*(~Z)*c0 - c1) * (~Z);  1/Z ~= -c0*w1   (~0.2% rel err)
    with -c0/16 folded into the zb broadcast selector. All tiles live on
    partitions 64-79 to stay aligned with the PSUM Z rows.
  - zb broadcast via K=16 selector matmul to 64 partitions; ha = ha_u*zb.
  - PE filler between S_T chunks: v / next-pair qk / prev-pair AV / zb.
  - proj: (psum + b_proj) + x fused in one DVE scalar_tensor_tensor; first
    two output tiles defer their last cc chunk so proj matmuls cover the
    final pair's normalization chain.
"""

import os

import numpy as np
import ml_dtypes

import concourse.bass as bass
import concourse.bacc as bacc
import concourse.mybir as mybir
import concourse.tile as tile
from concourse.bass_utils import run_bass_kernel_spmd

F32 = mybir.dt.float32
BF16 = mybir.dt.bfloat16
I32 = mybir.dt.int32
AF = mybir.ActivationFunctionType
ALU = mybir.AluOpType

B = 8
C = 512
N = 1024          # H*W = 32*32
H = 8             # num heads
HD = 64           # head dim
G = 32            # groups
GS = C // G       # channels per group = 16
CCH = 4           # channel chunks of 128
NT = 2            # n tiles of 512
MT = 8            # m tiles of 128
EPS = 1e-5
P = 128
NCORES = 8
W80 = HD + 16     # v columns + 16 ones columns per head

RC0 = -0.23549792          # Chebyshev recip seed scale
RC1 = 2.0017324
RSQRT_MAGIC = 0x5F3759DF

_CACHE = {}


def build_nc():
    nc = bacc.Bacc(
        "TRN2", target_bir_lowering=False, debug=False, num_devices=NCORES
    )

    x_d = nc.declare_dram_parameter("x", [C, N], F32, isOutput=False)
    wqk_d = nc.declare_dram_parameter("w_qkT", [P, CCH * 2 * C], BF16, isOutput=False)
    wv_d = nc.declare_dram_parameter("w_vT", [P, CCH * C], BF16, isOutput=False)
    wvb_d = nc.declare_dram_parameter("w_vb", [1, C], BF16, isOutput=False)
    wp_d = nc.declare_dram_parameter("w_pT", [P, CCH * C], BF16, isOutput=False)
    pf_d = nc.declare_dram_parameter("pf32", [P, 16], F32, isOutput=False)
    pb_d = nc.declare_dram_parameter("pbf16", [P, 200], BF16, isOutput=False)
    out_d = nc.declare_dram_parameter("out", [C, N], F32, isOutput=True)

    with tile.TileContext(nc) as tc:
        with (
            tc.tile_pool(name="singles", bufs=1) as singles,
            tc.tile_pool(name="outbuf", bufs=2) as outbuf,
            tc.tile_pool(name="ps", bufs=2, space="PSUM") as ps_pool,
            tc.tile_pool(name="ps_av", bufs=2, space="PSUM") as ps_av_pool,
        ):
            # ---------------- static tiles ----------------
            x_sb = singles.tile([P, CCH, N], F32)
            xn_sb = singles.tile([P, CCH, N], BF16)
            wqk_sb = singles.tile([P, CCH, 2 * C], BF16)
            wv_sb = singles.tile([P, CCH, C], BF16)
            wvb_sb = singles.tile([1, C], BF16)
            wp_sb = singles.tile([P, CCH, C], BF16)
            pf_sb = singles.tile([P, 16], F32)    # bq(4) bp(4) gamma(4) beta(4)
            pb_sb = singles.tile([P, 200], BF16)  # gsel(8) gselT(128) zsel(64)

            qk_sb = singles.tile([P, 8, N], BF16)       # ot 0-3: q, 4-7: k
            vT_sb = singles.tile([P, MT, H * W80], BF16)
            pT_t = [
                singles.tile([P, MT, N], BF16, name=f"pT{i}") for i in range(4)
            ]
            ha_sb = singles.tile([P, CCH, N], BF16)
            ones_row = singles.tile([1, P], BF16)

            # GroupNorm scratch
            s12_sb = singles.tile([P, 8], F32)          # cols 0-3 sum, 4-7 sumsq
            s12_bf = singles.tile([P, 8], BF16)
            sq_scr = singles.tile([P, N], F32)
            mu_rs = singles.tile([8, 8], F32)           # cols 0-3 mu, 4-7 rs
            mu_rs_bf = singles.tile([8, 8], BF16)
            ex2_sb = singles.tile([8, CCH], F32)
            tmp8 = singles.tile([8, CCH], F32)
            var_sb = singles.tile([8, CCH], F32)        # then var+eps
            rsq_i = singles.tile([8, CCH], I32)         # int seed scratch
            rsq_r = singles.tile([8, CCH], F32)         # rsqrt iterate
            rsq_t = singles.tile([8, CCH], F32)
            rsq_u = singles.tile([8, CCH], F32)
            s0_sb = singles.tile([P, CCH], F32)
            sbias_sb = singles.tile([P, CCH], F32)
            tmp128 = singles.tile([P, CCH], F32)

            # softmax 1/Z scratch: rows 64-79 only (aligned with PSUM Z rows);
            # col range [0, N) = head A of the pair, [N, 2N) = head B.
            zw_sb = singles.tile([P, 2 * N], F32)       # ~Z seed (bits)
            zt_sb = singles.tile([P, 2 * N], F32)       # Z * y0
            zi_sb = singles.tile([P, 2 * N], BF16)      # w1 (recip * -1/c0)
            zb_t = [singles.tile([HD, N], F32, name=f"zb{i}") for i in range(4)]

            # ---------------- input DMAs (3 queues) ----------------
            x_v = x_d.ap().rearrange("(cc p) n -> p cc n", p=P)
            for cc in range(CCH):
                nc.sync.dma_start(x_sb[:, cc, :], x_v[:, cc, :])
            nc.scalar.dma_start(
                wqk_sb[:], wqk_d.ap().rearrange("p (cc o) -> p cc o", cc=CCH)
            )
            nc.scalar.dma_start(
                wv_sb[:], wv_d.ap().rearrange("p (cc o) -> p cc o", cc=CCH)
            )
            nc.scalar.dma_start(wvb_sb[:], wvb_d.ap())
            nc.scalar.dma_start(
                wp_sb[:], wp_d.ap().rearrange("p (cc o) -> p cc o", cc=CCH)
            )
            nc.gpsimd.dma_start(pf_sb[:], pf_d.ap())
            nc.gpsimd.dma_start(pb_sb[:], pb_d.ap())

            bq = pf_sb[:, 0:4]
            bp = pf_sb[:, 4:8]
            gm = pf_sb[:, 8:12]
            bt = pf_sb[:, 12:16]
            gsel = pb_sb[:, 0:8]
            gselT = pb_sb[0:8, 8:136]
            zsel = pb_sb[64:80, 136:200]   # [16, 64] = -c0/16

            nc.vector.memset(ones_row[:], 1.0)
            nc.vector.memset(
                vT_sb[:].rearrange("p mt (h d) -> p mt h d", h=H)[:, :, :, HD:W80],
                1.0,
            )
            # Preload the exp ACT table set while the input DMAs run.
            nc.scalar.activation(sq_scr[0:1, 0:P], ones_row[:], AF.Exp)

            # ---------------- GroupNorm stats ----------------
            for cc in range(CCH):
                nc.vector.tensor_scalar(
                    out=sq_scr[:], in0=x_sb[:, cc, :],
                    scalar1=1.0, scalar2=None, op0=ALU.mult, op1=ALU.add,
                    accum_out=s12_sb[:, cc : cc + 1],
                )
                nc.vector.scalar_tensor_tensor(
                    out=sq_scr[:],
                    in0=x_sb[:, cc, :],
                    scalar=1.0,
                    in1=x_sb[:, cc, :],
                    op0=ALU.mult,
                    op1=ALU.mult,
                    accum_out=s12_sb[:, 4 + cc : 5 + cc],
                )
            nc.vector.tensor_copy(s12_bf[:], s12_sb[:])
            ps_st = ps_pool.tile([P, N], F32, tag="ps")
            nc.tensor.matmul(
                ps_st[0:8, 0:8], gsel, s12_bf[:], start=True, stop=True
            )
            inv_cnt = 1.0 / (GS * N)
            nc.vector.tensor_scalar_mul(mu_rs[:, 0:4], ps_st[0:8, 0:4], inv_cnt)
            nc.vector.tensor_scalar_mul(ex2_sb[:], ps_st[0:8, 4:8], inv_cnt)
            nc.vector.tensor_mul(tmp8[:], mu_rs[:, 0:4], mu_rs[:, 0:4])
            nc.vector.tensor_sub(var_sb[:], ex2_sb[:], tmp8[:])
            nc.vector.tensor_scalar_add(var_sb[:], var_sb[:], EPS)
            # rsqrt(var+eps): int seed MAGIC - (bits >> 1), then 2 Newton steps
            nc.vector.tensor_scalar(
                out=rsq_i[:], in0=var_sb[:].bitcast(I32),
                scalar1=1, scalar2=None, op0=ALU.arith_shift_right,
            )
            nc.vector.tensor_scalar_sub(rsq_i[:], rsq_i[:], RSQRT_MAGIC)
            nc.vector.tensor_scalar(            # ~x (then +1 below: -x = ~x+1)
                out=rsq_i[:], in0=rsq_i[:],
                scalar1=0, scalar2=None, op0=ALU.bitwise_not,
            )
            nc.vector.tensor_scalar_add(rsq_r[:].bitcast(I32), rsq_i[:], 1)
            for _ in range(1):
                nc.vector.tensor_mul(rsq_t[:], rsq_r[:], rsq_r[:])
                nc.vector.scalar_tensor_tensor(
                    out=rsq_u[:], in0=rsq_t[:], scalar=-0.5, in1=var_sb[:],
                    op0=ALU.mult, op1=ALU.mult,
                )
                nc.vector.scalar_tensor_tensor(
                    out=rsq_r[:], in0=rsq_u[:], scalar=1.5, in1=rsq_r[:],
                    op0=ALU.add, op1=ALU.mult,
                )
            ps_wu = ps_pool.tile([P, N], F32, tag="ps", name="warmup")
            for wu in range(24):
                nc.tensor.matmul(
                    ps_wu[:, (wu % 2) * 512 : (wu % 2) * 512 + 512],
                    wqk_sb[:, 0, 0:P],
                    wqk_sb[:, 1, 0:512],
                    start=True, stop=True,
                )
            nc.vector.tensor_copy(mu_rs[:, 4:8], rsq_r[:])
            nc.vector.tensor_copy(mu_rs_bf[:], mu_rs[:])
            ps_bc = ps_pool.tile([P, N], F32, tag="ps")
            nc.tensor.matmul(
                ps_bc[0:P, 0:8], gselT, mu_rs_bf[:], start=True, stop=True
            )
            nc.vector.tensor_mul(s0_sb[:], ps_bc[0:P, 4:8], gm)
            nc.vector.tensor_mul(tmp128[:], ps_bc[0:P, 0:4], s0_sb[:])
            nc.vector.tensor_sub(sbias_sb[:], bt, tmp128[:])
            for cc in range(CCH):
                nc.vector.tensor_scalar(
                    out=xn_sb[:, cc, :],
                    in0=x_sb[:, cc, :],
                    scalar1=s0_sb[:, cc : cc + 1],
                    scalar2=sbias_sb[:, cc : cc + 1],
                    op0=ALU.mult,
                    op1=ALU.add,
                )

            # ---------------- emission helpers ----------------
            def qk_ot(ot):
                """q (ot<4) or k (ot>=4) output tile: 8 matmuls + eviction."""
                ps_qk = ps_pool.tile([P, N], F32, tag="ps", name=f"qk{ot}")
                for cc in range(CCH):
                    for nt in range(NT):
                        nc.tensor.matmul(
                            ps_qk[:, nt * 512 : (nt + 1) * 512],
                            wqk_sb[:, cc, ot * P : (ot + 1) * P],
                            xn_sb[:, cc, nt * 512 : (nt + 1) * 512],
                            start=(cc == 0),
                            stop=(cc == CCH - 1),
                        )
                if ot < 4:  # q: add bias on eviction
                    nc.vector.tensor_scalar_add(
                        qk_sb[:, ot, :], ps_qk[:], bq[:, ot : ot + 1]
                    )
                else:       # k: bias cancels in softmax; plain copy
                    nc.vector.tensor_copy(qk_sb[:, ot, :], ps_qk[:])

            def v_mt(mt):
                """v_T chunk for rows [128*mt, 128*mt+128): 5 matmuls + copy."""
                ps_v = ps_pool.tile([P, N], F32, tag="ps", name=f"v{mt}")
                for cc in range(CCH):
                    nc.tensor.matmul(
                        ps_v[:, 0:C],
                        xn_sb[:, cc, mt * P : (mt + 1) * P],
                        wv_sb[:, cc, :],
                        start=(cc == 0),
                        stop=False,
                    )
                nc.tensor.matmul(
                    ps_v[:, 0:C], ones_row[:], wvb_sb[:], start=False, stop=True
                )
                nc.scalar.copy(
                    vT_sb[:, mt, :]
                    .rearrange("p (h d) -> p h d", h=H)[:, :, 0:HD],
                    ps_v[:, 0:C].rearrange("p (h d) -> p h d", h=H),
                )

            def st_pair_mt(pr, mt):
                """S_T + exp for heads (2pr, 2pr+1), m-chunk mt; disjoint PE
                row groups -> the two heads' matmuls can run concurrently."""
                ot = pr
                pa = pT_t[(pr % 2) * 2]
                pb2 = pT_t[(pr % 2) * 2 + 1]
                psA = ps_pool.tile([P, N], F32, tag="ps", name=f"sA{pr}_{mt}")
                psB = ps_pool.tile([P, N], F32, tag="ps", name=f"sB{pr}_{mt}")
                for nt in range(NT):
                    sl = slice(nt * 512, (nt + 1) * 512)
                    nc.tensor.matmul(
                        psB[:, sl],
                        qk_sb[HD:P, 4 + ot, mt * P : (mt + 1) * P],
                        qk_sb[HD:P, ot, sl],
                        start=True, stop=True,
                    )
                    nc.tensor.matmul(
                        psA[:, sl],
                        qk_sb[0:HD, 4 + ot, mt * P : (mt + 1) * P],
                        qk_sb[0:HD, ot, sl],
                        start=True, stop=True,
                    )
                scale = float(HD) ** -0.5
                nc.scalar.activation(pa[:, mt, :], psA[:], AF.Exp, scale=scale)
                nc.scalar.activation(pb2[:, mt, :], psB[:], AF.Exp, scale=scale)

            def av_mt(pr, mt, ps_avA, ps_avB):
                """AV accumulation chunk mt for head pair pr (4 matmuls)."""
                hA, hB = 2 * pr, 2 * pr + 1
                pa = pT_t[(pr % 2) * 2]
                pb2 = pT_t[(pr % 2) * 2 + 1]
                for (h, pt, ps_av) in ((hA, pa, ps_avA), (hB, pb2, ps_avB)):
                    for nt in range(NT):
                        sl = slice(nt * 512, (nt + 1) * 512)
                        nc.tensor.matmul(
                            ps_av[0:W80, sl],
                            vT_sb[:, mt, h * W80 : (h + 1) * W80],
                            pt[:, mt, sl],
                            start=(mt == 0),
                            stop=(mt == MT - 1),
                        )

            def norm_pair_dve(pr, ps_avA, ps_avB):
                """w1 = (Z*~Z*c0 - c1)*~Z per head (fast recip minus the final
                scale, folded into zsel). Standard DVE ops only."""
                for (ps_av, off) in ((ps_avA, 0), (ps_avB, N)):
                    zr = ps_av[HD:W80, :]
                    sl = slice(off, off + N)
                    nc.vector.tensor_scalar(
                        out=zw_sb[HD:W80, sl].bitcast(I32),
                        in0=zr.bitcast(I32),
                        scalar1=0, scalar2=None, op0=ALU.bitwise_not,
                    )
                    nc.vector.scalar_tensor_tensor(   # zt = (~Z * c0) * Z
                        out=zt_sb[HD:W80, sl],
                        in0=zw_sb[HD:W80, sl],
                        scalar=RC0,
                        in1=zr,
                        op0=ALU.mult, op1=ALU.mult,
                    )
                    nc.vector.scalar_tensor_tensor(   # w1 = (zt - c1) * ~Z
                        out=zi_sb[HD:W80, sl],
                        in0=zt_sb[HD:W80, sl],
                        scalar=RC1,
                        in1=zw_sb[HD:W80, sl],
                        op0=ALU.subtract, op1=ALU.mult,
                    )

            def norm_pair_pe(pr, ps_avA, ps_avB):
                """zb = broadcast(1/Z) to 64 partitions (selector carries the
                -c0/16 scale), then ha = ha_u * zb on DVE."""
                ot = pr
                zbA = zb_t[(pr % 2) * 2]
                zbB = zb_t[(pr % 2) * 2 + 1]
                ps_zbA = ps_pool.tile([P, N], F32, tag="ps", name=f"zbA{pr}")
                ps_zbB = ps_pool.tile([P, N], F32, tag="ps", name=f"zbB{pr}")
                for nt in range(NT):
                    sl = slice(nt * 512, (nt + 1) * 512)
                    nc.tensor.matmul(
                        ps_zbA[0:HD, sl], zsel, zi_sb[HD:W80, sl],
                        start=True, stop=True,
                    )
                    nc.tensor.matmul(
                        ps_zbB[0:HD, sl], zsel,
                        zi_sb[HD:W80, N + sl.start : N + sl.stop],
                        start=True, stop=True,
                    )
                nc.scalar.copy(zbA[:], ps_zbA[0:HD, :])
                nc.scalar.copy(zbB[:], ps_zbB[0:HD, :])
                nc.vector.tensor_mul(ha_sb[0:HD, ot, :], ps_avA[0:HD, :], zbA[:])
                nc.vector.tensor_mul(ha_sb[HD:P, ot, :], ps_avB[0:HD, :], zbB[:])

            def proj_ot_mms(ot, ps_p, ccs):
                for cc in ccs:
                    for nt in range(NT):
                        nc.tensor.matmul(
                            ps_p[:, nt * 512 : (nt + 1) * 512],
                            wp_sb[:, cc, ot * P : (ot + 1) * P],
                            ha_sb[:, cc, nt * 512 : (nt + 1) * 512],
                            start=(cc == 0),
                            stop=(cc == CCH - 1),
                        )

            # ---------------- prologue: q0/k0 ----------------
            qk_ot(0)
            qk_ot(4)

            # ---------------- head-pair pipeline ----------------
            # Step 0: S_T/exp of pair 0, with v chunks and q1/k1 as PE filler.
            for mt in range(MT):
                st_pair_mt(0, mt)
                v_mt(mt)
                if mt == 2:
                    qk_ot(1)
                if mt == 5:
                    qk_ot(5)

            # Steps 1-3: S_T/exp of pair pr overlapped with AV of pair pr-1
            # (AV one chunk ahead so Z completes while S_T still has work).
            for pr in range(1, 4):
                avA = ps_av_pool.tile([P, N], F32, tag="av", name=f"avA{pr-1}")
                avB = ps_av_pool.tile([P, N], F32, tag="av", name=f"avB{pr-1}")
                st_pair_mt(pr, 0)
                av_mt(pr - 1, 0, avA, avB)
                av_mt(pr - 1, 1, avA, avB)
                for mt in range(1, MT - 1):
                    st_pair_mt(pr, mt)
                    av_mt(pr - 1, mt + 1, avA, avB)
                    if mt == MT - 2:
                        norm_pair_dve(pr - 1, avA, avB)
                    if pr < 3:
                        if mt == 1:
                            qk_ot(pr + 1)
                        if mt == 3:
                            qk_ot(pr + 5)
                st_pair_mt(pr, MT - 1)
                norm_pair_pe(pr - 1, avA, avB)

            # Final step: AV of pair 3, normalization, then proj with the
            # first two output tiles deferring their last cc chunk.
            avA = ps_av_pool.tile([P, N], F32, tag="av", name="avA3")
            avB = ps_av_pool.tile([P, N], F32, tag="av", name="avB3")
            for mt in range(MT):
                av_mt(3, mt, avA, avB)
            norm_pair_dve(3, avA, avB)
            norm_pair_pe(3, avA, avB)

            # ---------------- proj + bias + residual ----------------
            out_v = out_d.ap().rearrange("(ot p) n -> p ot n", p=P)

            def proj_evict(ot, ps_p):
                out_t = outbuf.tile([P, N], F32, tag="out")
                nc.vector.scalar_tensor_tensor(
                    out=out_t[:],
                    in0=ps_p[:],
                    scalar=bp[:, ot : ot + 1],
                    in1=x_sb[:, ot, :],
                    op0=ALU.add,
                    op1=ALU.add,
                )
                nc.sync.dma_start(out_v[:, ot, :], out_t[:])

            ps_p0 = ps_pool.tile([P, N], F32, tag="ps", name="proj0")
            proj_ot_mms(0, ps_p0, range(CCH - 1))
            ps_p1 = ps_pool.tile([P, N], F32, tag="ps", name="proj1")
            proj_ot_mms(1, ps_p1, range(CCH - 1))
            proj_ot_mms(0, ps_p0, [CCH - 1])
            proj_evict(0, ps_p0)
            proj_ot_mms(1, ps_p1, [CCH - 1])
            proj_evict(1, ps_p1)
            for ot in range(2, CCH):
                ps_p = ps_pool.tile([P, N], F32, tag="ps", name=f"proj{ot}")
                proj_ot_mms(ot, ps_p, range(CCH))
                proj_evict(ot, ps_p)

    nc.compile()
    return nc


def make_in_maps(x, gn_gamma, gn_beta, w_qkv, b_qkv, w_proj, b_proj):
    f32 = np.float32
    bf16 = ml_dtypes.bfloat16
    w_qkv = np.asarray(w_qkv, dtype=f32)
    b_qkv = np.asarray(b_qkv, dtype=f32)

    def chunked_T(w):
        # [O, C_in] -> transposed [C_in, O] -> SBUF layout [p, cc, O] packed
        # as [P, cc*O] with in-channel c = cc*128 + p.
        wt = np.ascontiguousarray(w.T, dtype=f32)  # [C_in, O]
        o = wt.shape[1]
        return np.ascontiguousarray(
            wt.reshape(CCH, P, o).transpose(1, 0, 2).reshape(P, CCH * o)
        ).astype(bf16)

    def perch(v):
        return np.asarray(v, dtype=f32).reshape(CCH, P).T  # [P, CCH]

    pf32 = np.concatenate(
        [
            perch(b_qkv[:C]),          # q bias
            perch(np.asarray(b_proj, dtype=f32)),
            perch(np.asarray(gn_gamma, dtype=f32)),
            perch(np.asarray(gn_beta, dtype=f32)),
        ],
        axis=1,
    ).astype(f32)

    # pbf16: gsel [P, 0:8], gselT [rows 0-7, 8:136], zsel [rows 64-79, 136:200]
    gsel = np.zeros((P, 8), f32)
    for p in range(P):
        gsel[p, p // GS] = 1.0
    pbf16 = np.zeros((P, 200), f32)
    pbf16[:, 0:8] = gsel
    pbf16[0:8, 8:136] = gsel.T
    pbf16[64:80, 136:200] = -RC0 / 16.0
    pbf16 = np.ascontiguousarray(pbf16).astype(bf16)

    shared = {
        "w_qkT": chunked_T(w_qkv[: 2 * C]),
        "w_vT": chunked_T(w_qkv[2 * C :]),
        "w_vb": np.ascontiguousarray(b_qkv[2 * C :][None, :]).astype(bf16),
        "w_pT": chunked_T(np.asarray(w_proj, dtype=f32)),
        "pf32": pf32,
        "pbf16": pbf16,
    }
    in_maps = []
    for b in range(B):
        m = dict(shared)
        m["x"] = np.ascontiguousarray(np.asarray(x[b], dtype=f32).reshape(C, N))
        in_maps.append(m)
    return in_maps


def kernel(x, gn_gamma, gn_beta, w_qkv, b_qkv, w_proj, b_proj):
    if "nc" not in _CACHE:
        _CACHE["nc"] = build_nc()
    nc = _CACHE["nc"]
    in_maps = make_in_maps(x, gn_gamma, gn_beta, w_qkv, b_qkv, w_proj, b_proj)
    trace = bool(os.environ.get("KERNEL_TRACE"))
    res = run_bass_kernel_spmd(
        nc, in_maps, core_ids=list(range(NCORES)), trace=trace
    )
    _CACHE["last_result"] = res
    out = np.stack([np.asarray(res.results[i]["out"]) for i in range(NCORES)])
    return out.reshape(B, C, 32, 32).astype(np.float32)
